# revision 15
# baseline (speedup 1.0000x reference)
"""Strided (stride=1) valid 1D conv on Trainium2, data-parallel over batch.

Problem: x (16, 32, 32768) f32, kernel (1, 32, 32, 3) f32
         -> out (16, 32, 32766) f32  (valid conv, NCH / OIH layout)

Strategy (per core, 2 batches each across 8 cores):
  out[b, co, l] = sum_{ci,k} W[co, ci, k] * x[b, ci, l + k]

  Channel count is 32, so we pack 4 independent L-chunks ("groups") into
  the 128 SBUF partitions: partition (g*32 + ci) holds x[b, ci, base+g*G+j].
  A block-diagonal [128, 128] weight matrix (4 copies of W_k^T on the
  diagonal) turns the 4-group conv tap into ONE K=128 matmul.  The 3 taps
  accumulate into one PSUM bank (start/stop flags).  Operands are typed
  float32r for the fast PE path (1 cycle/row at N=512 vs 4 for fp32).

  x is host-padded by 2 columns and the output is computed padded to
  32768 columns so the tiling is perfectly regular (no ragged tail, no
  overlapping output writes); the host slices the pad off.

  Raw Bass (not Tile): walrus codegen in this toolchain embeds at most
  ONE sync wait per Matmult / HWDGE DMACopy / Drain, which Tile's
  auto-generated semaphores routinely exceed.  Here every cross-engine
  wait is an explicit standalone wait_ge on the engine's sequencer:
    sync   : weight DMA + 4 input DMAs up front, then per-tile output
             DMAs gated on that tile's PSUM-drain count.
    tensor : per tile, 8 groups x 3 accumulating fp32r matmuls; bank j
             is reused across tiles, gated on the drain of its previous
             occupant; last matmul of a group bumps sem_mm.
    scalar : drains PSUM->SBUF for even tiles (ACT engine).
    vector : drains PSUM->SBUF for odd tiles (DVE engine).
"""

import sys

if "/opt/trn_rl_repo" not in sys.path:
    sys.path.insert(0, "/opt/trn_rl_repo")

from contextlib import ExitStack

import numpy as np

import concourse.bass as bass
import concourse.mybir as mybir
from concourse.ap import AP
from concourse.bass_utils import run_bass_kernel_spmd

# Problem shape (hardcoded; harness contract)
B, C, L = 16, 32, 32768
CO, KT = 32, 3
LOUT = L - KT + 1  # 32766
NCORES = 8
BPC = B // NCORES  # batches per core = 2

# Padded shapes used on-device
LP = L + 2      # x padded with 2 trailing zero columns
LOP = L         # output computed padded to 32768 (last 2 cols garbage)

# Tiling
NG = 4              # L-groups packed across the 128 partitions
G = 4096            # columns per group per tile
TILE_L = NG * G     # 16384 output cols per tile
NT = LOP // TILE_L  # tiles per batch = 2
NJ = G // 512       # 512-wide matmul chunks per group = 8
NTILES = BPC * NT   # 4

_CACHE = {}


def _cp_sem_count(ti: int, j: int) -> int:
    """Value of the drain engine's sem after copy (ti, j) completes.

    ACT drains even tiles, DVE odd tiles; each engine's sem counts its
    own copies in order.
    """
    return NJ * (ti // 2) + j + 1


def _build_nc():
    f32 = mybir.dt.float32
    f32r = mybir.dt.float32r

    nc = bass.Bass(trn_type="TRN2", target_bir_lowering=False)
    x = nc.dram_tensor("x", [BPC, C, LP], f32r, kind="ExternalInput")
    w = nc.dram_tensor("w", [KT, 128, 128], f32r, kind="ExternalInput")
    out = nc.dram_tensor("out", [BPC, CO, LOP], f32, kind="ExternalOutput")

    with ExitStack() as ctx:
        wt = ctx.enter_context(nc.sbuf_tensor("wt", [128, KT * 128], f32r))
        xts = [
            ctx.enter_context(nc.sbuf_tensor(f"xt{t}", [128, G + 2], f32r))
            for t in range(NTILES)
        ]
        osbs = [
            ctx.enter_context(nc.sbuf_tensor(f"osb{t}", [128, G], f32))
            for t in range(NTILES)
        ]
        psums = [
            ctx.enter_context(nc.psum_tensor(f"ps{j}", [128, 512], f32))
            for j in range(NJ)
        ]
        sem_w = ctx.enter_context(nc.semaphore("sem_w"))
        sem_x = ctx.enter_context(nc.semaphore("sem_x"))
        sem_mm = ctx.enter_context(nc.semaphore("sem_mm"))
        sem_cpa = ctx.enter_context(nc.semaphore("sem_cpa"))
        sem_cpb = ctx.enter_context(nc.semaphore("sem_cpb"))
        sem_out = ctx.enter_context(nc.semaphore("sem_out"))
        block = ctx.enter_context(nc.Block())

        @block.sync
        def _(sync):
            sync.dma_start(
                out=wt[:], in_=w.ap().rearrange("k p m -> p k m")
            ).then_inc(sem_w, 16)
            # Chain DMA issues on completion.  Multiple in-flight DMAs
            # bumping one counting sem complete engine-by-engine, so
            # "sem >= 16*(t+1)" would not imply DMA t finished; chaining
            # makes the counts exact (and satisfies CoreSim's race
            # detector).  Costs only issue latency: HBM bandwidth is the
            # shared bottleneck for these 9 transfers regardless.
            sync.wait_ge(sem_w, 16)
            for ti in range(NTILES):
                b, t = divmod(ti, NT)
                if ti > 0:
                    sync.wait_ge(sem_x, 16 * ti)
                src = AP(
                    x.ap().tensor,
                    b * C * LP + t * TILE_L,
                    [[G, NG], [LP, C], [1, G + 2]],
                )
                sync.dma_start(out=xts[ti][:], in_=src).then_inc(sem_x, 16)
            for ti in range(NTILES):
                b, t = divmod(ti, NT)
                sem_cp = sem_cpa if ti % 2 == 0 else sem_cpb
                sync.wait_ge(sem_cp, _cp_sem_count(ti, NJ - 1))
                if ti > 0:
                    sync.wait_ge(sem_out, 16 * ti)
                dstap = AP(
                    out.ap().tensor,
                    b * CO * LOP + t * TILE_L,
                    [[G, NG], [LOP, CO], [1, G]],
                )
                sync.dma_start(out=dstap, in_=osbs[ti][:]).then_inc(
                    sem_out, 16
                )
            sync.wait_ge(sem_out, 16 * NTILES)

        @block.tensor
        def _(tensor):
            tensor.wait_ge(sem_w, 16)
            for ti in range(NTILES):
                tensor.wait_ge(sem_x, 16 * (ti + 1))
                for j in range(NJ):
                    if ti > 0:
                        # bank j was drained by the previous tile's engine
                        prev_sem = sem_cpa if (ti - 1) % 2 == 0 else sem_cpb
                        tensor.wait_ge(prev_sem, _cp_sem_count(ti - 1, j))
                    mm = None
                    for k in range(KT):
                        mm = tensor.matmul(
                            psums[j][:],
                            wt[:, k * 128 : (k + 1) * 128],
                            xts[ti][:, j * 512 + k : j * 512 + k + 512],
                            start=(k == 0),
                            stop=(k == KT - 1),
                        )
                    mm.then_inc(sem_mm, 1)

        @block.scalar
        def _(scalar):
            for ti in range(0, NTILES, 2):
                for j in range(NJ):
                    scalar.wait_ge(sem_mm, ti * NJ + j + 1)
                    scalar.copy(
                        osbs[ti][:, j * 512 : (j + 1) * 512], psums[j][:]
                    ).then_inc(sem_cpa, 1)

        @block.vector
        def _(vector):
            for ti in range(1, NTILES, 2):
                for j in range(NJ):
                    vector.wait_ge(sem_mm, ti * NJ + j + 1)
                    vector.tensor_copy(
                        osbs[ti][:, j * 512 : (j + 1) * 512], psums[j][:]
                    ).then_inc(sem_cpb, 1)

    return nc


def _block_diag_weights(kernel: np.ndarray) -> np.ndarray:
    """kernel (1, CO, C, KT) -> (KT, 128, 128) block-diag lhsT.

    lhsT[k][ci + 32*g, co + 32*g] = kernel[0, co, ci, k]
    """
    wbd = np.zeros((KT, 128, 128), dtype=np.float32)
    wt = np.ascontiguousarray(kernel[0].transpose(2, 1, 0))  # (KT, C, CO)
    for g in range(NG):
        wbd[:, g * 32 : (g + 1) * 32, g * 32 : (g + 1) * 32] = wt
    return wbd


def _pad_x(x: np.ndarray) -> np.ndarray:
    xp = np.zeros((B, C, LP), dtype=np.float32)
    xp[:, :, :L] = x
    return xp


def kernel(x: np.ndarray, kernel: np.ndarray) -> np.ndarray:
    if "nc" not in _CACHE:
        _CACHE["nc"] = _build_nc()
    nc = _CACHE["nc"]

    wbd = _block_diag_weights(np.asarray(kernel, dtype=np.float32))
    xp = _pad_x(np.asarray(x, dtype=np.float32))

    in_maps = [
        {"x": xp[i * BPC : (i + 1) * BPC], "w": wbd} for i in range(NCORES)
    ]
    res = run_bass_kernel_spmd(nc, in_maps, list(range(NCORES)))
    full = np.concatenate([r["out"] for r in res.results], axis=0)
    return np.ascontiguousarray(full[:, :, :LOUT])


# revision 17
# speedup vs baseline: 41.8561x; 41.8561x over previous
"""Strided (stride=1) valid 1D conv on Trainium2, data-parallel over batch.

Problem: x (16, 32, 32768) f32, kernel (1, 32, 32, 3) f32
         -> out (16, 32, 32766) f32  (valid conv, NCH / OIH layout)

Strategy (per core, 2 batches each across 8 cores):
  out[b, co, l] = sum_{ci,k} W[co, ci, k] * x[b, ci, l + k]

  Channel count is 32, so we pack 4 independent L-chunks ("groups") into
  the 128 SBUF partitions: partition (g*32 + ci) holds x[b, ci, base+g*G+j].
  A block-diagonal [128, 128] weight matrix (4 copies of W_k^T on the
  diagonal) turns the 4-group conv tap into ONE K=128 matmul.  The 3 taps
  accumulate into one PSUM bank (start/stop flags).  Operands are typed
  float32r for the fast PE path (1 cycle/row at N=512 vs 4 for fp32).

  x is host-padded by 2 columns and the output is computed padded to
  32768 columns so the tiling is perfectly regular (no ragged tail, no
  overlapping output writes); the host slices the pad off.

  Raw Bass (not Tile): walrus codegen in this toolchain embeds at most
  ONE sync wait per Matmult / HWDGE DMACopy / Drain, which Tile's
  auto-generated semaphores routinely exceed.  Here every cross-engine
  wait is an explicit standalone wait_ge on the engine's sequencer:
    sync   : weight DMA + 4 input DMAs up front, then per-tile output
             DMAs gated on that tile's PSUM-drain count.
    tensor : per tile, 8 groups x 3 accumulating fp32r matmuls; bank j
             is reused across tiles, gated on the drain of its previous
             occupant; last matmul of a group bumps sem_mm.
    scalar : drains PSUM->SBUF for even tiles (ACT engine).
    vector : drains PSUM->SBUF for odd tiles (DVE engine).
"""

import sys

if "/opt/trn_rl_repo" not in sys.path:
    sys.path.insert(0, "/opt/trn_rl_repo")

from contextlib import ExitStack

import numpy as np

import concourse.bass as bass
import concourse.mybir as mybir
from concourse.ap import AP
from concourse.bass_utils import run_bass_kernel_spmd

# Problem shape (hardcoded; harness contract)
B, C, L = 16, 32, 32768
CO, KT = 32, 3
LOUT = L - KT + 1  # 32766
NCORES = 8
BPC = B // NCORES  # batches per core = 2

# Padded shapes used on-device
LP = L + 2      # x padded with 2 trailing zero columns
LOP = L         # output computed padded to 32768 (last 2 cols garbage)

# Tiling
NG = 4              # L-groups packed across the 128 partitions
G = 4096            # columns per group per tile
TILE_L = NG * G     # 16384 output cols per tile
NT = LOP // TILE_L  # tiles per batch = 2
NJ = G // 512       # 512-wide matmul chunks per group = 8
NTILES = BPC * NT   # 4

_CACHE = {}


def _cp_sem_count(gt: int, j: int) -> int:
    """Value of the drain engine's sem after copy (gt, j) completes.

    ACT drains even global tiles, DVE odd ones; each engine's sem
    counts its own copies in order.  gt is the global tile index
    (rep * NTILES + ti); reps only exist for benchmarking.
    """
    return NJ * (gt // 2) + j + 1


def _build_nc(reps: int = 1):
    f32 = mybir.dt.float32
    f32r = mybir.dt.float32r

    nc = bass.Bass(trn_type="TRN2", target_bir_lowering=False)
    x = nc.dram_tensor("x", [BPC, C, LP], f32r, kind="ExternalInput")
    w = nc.dram_tensor("w", [KT, 128, 128], f32r, kind="ExternalInput")
    out = nc.dram_tensor("out", [BPC, CO, LOP], f32, kind="ExternalOutput")

    with ExitStack() as ctx:
        wt = ctx.enter_context(nc.sbuf_tensor("wt", [128, KT * 128], f32r))
        xts = [
            ctx.enter_context(nc.sbuf_tensor(f"xt{t}", [128, G + 2], f32r))
            for t in range(NTILES)
        ]
        osbs = [
            ctx.enter_context(nc.sbuf_tensor(f"osb{t}", [128, G], f32))
            for t in range(NTILES)
        ]
        psums = [
            ctx.enter_context(nc.psum_tensor(f"ps{j}", [128, 512], f32))
            for j in range(NJ)
        ]
        sem_w = ctx.enter_context(nc.semaphore("sem_w"))
        sem_x = ctx.enter_context(nc.semaphore("sem_x"))
        sem_mm = ctx.enter_context(nc.semaphore("sem_mm"))
        sem_cpa = ctx.enter_context(nc.semaphore("sem_cpa"))
        sem_cpb = ctx.enter_context(nc.semaphore("sem_cpb"))
        sem_out = ctx.enter_context(nc.semaphore("sem_out"))
        block = ctx.enter_context(nc.Block())

        @block.sync
        def _(sync):
            sync.dma_start(
                out=wt[:], in_=w.ap().rearrange("k p m -> p k m")
            ).then_inc(sem_w, 16)
            # Chain DMA issues on completion.  Multiple in-flight DMAs
            # bumping one counting sem complete engine-by-engine, so
            # "sem >= 16*(t+1)" would not imply DMA t finished; chaining
            # makes the counts exact (and satisfies CoreSim's race
            # detector).  Costs only issue latency: HBM bandwidth is the
            # shared bottleneck for these 9 transfers regardless.
            sync.wait_ge(sem_w, 16)
            # Interleave input and output DMA issue (outs lag ins by
            # LAG tiles).  Issuing all ins before all outs deadlocks at
            # reps >= 3: a late in-DMA waits on compute whose PSUM
            # drain transitively needs an out-DMA queued behind it in
            # this same FIFO.
            TOT = NTILES * reps
            LAG = 2
            for gt in range(TOT + LAG):
                if gt < TOT:
                    ti = gt % NTILES
                    b, t = divmod(ti, NT)
                    if gt > 0:
                        sync.wait_ge(sem_x, 16 * gt)
                    if gt >= NTILES:
                        # xt slot reused: previous rep's reads done when
                        # all of that tile's matmul groups completed
                        sync.wait_ge(sem_mm, NJ * (gt - NTILES + 1))
                    src = AP(
                        x.ap().tensor,
                        b * C * LP + t * TILE_L,
                        [[G, NG], [LP, C], [1, G + 2]],
                    )
                    sync.dma_start(out=xts[ti][:], in_=src).then_inc(
                        sem_x, 16
                    )
                og = gt - LAG
                if og >= 0:
                    ti = og % NTILES
                    b, t = divmod(ti, NT)
                    sem_cp = sem_cpa if og % 2 == 0 else sem_cpb
                    sync.wait_ge(sem_cp, _cp_sem_count(og, NJ - 1))
                    if og > 0:
                        sync.wait_ge(sem_out, 16 * og)
                    dstap = AP(
                        out.ap().tensor,
                        b * CO * LOP + t * TILE_L,
                        [[G, NG], [LOP, CO], [1, G]],
                    )
                    sync.dma_start(out=dstap, in_=osbs[ti][:]).then_inc(
                        sem_out, 16
                    )
            sync.wait_ge(sem_out, 16 * TOT)

        @block.tensor
        def _(tensor):
            tensor.wait_ge(sem_w, 16)
            for r in range(reps):
                for ti in range(NTILES):
                    gt = r * NTILES + ti
                    tensor.wait_ge(sem_x, 16 * (gt + 1))
                    for j in range(NJ):
                        if gt > 0:
                            # bank j was drained by the previous global
                            # tile's engine
                            prev_sem = (
                                sem_cpa if (gt - 1) % 2 == 0 else sem_cpb
                            )
                            tensor.wait_ge(prev_sem, _cp_sem_count(gt - 1, j))
                        mm = None
                        for k in range(KT):
                            mm = tensor.matmul(
                                psums[j][:],
                                wt[:, k * 128 : (k + 1) * 128],
                                xts[ti][:, j * 512 + k : j * 512 + k + 512],
                                start=(k == 0),
                                stop=(k == KT - 1),
                            )
                        mm.then_inc(sem_mm, 1)

        @block.scalar
        def _(scalar):
            for gt in range(0, NTILES * reps, 2):
                ti = gt % NTILES
                for j in range(NJ):
                    scalar.wait_ge(sem_mm, gt * NJ + j + 1)
                    if gt >= NTILES and j == 0:
                        # osb slot reused: previous rep's out-DMA done
                        scalar.wait_ge(sem_out, 16 * (gt - NTILES + 1))
                    scalar.copy(
                        osbs[ti][:, j * 512 : (j + 1) * 512], psums[j][:]
                    ).then_inc(sem_cpa, 1)

        @block.vector
        def _(vector):
            for gt in range(1, NTILES * reps, 2):
                ti = gt % NTILES
                for j in range(NJ):
                    vector.wait_ge(sem_mm, gt * NJ + j + 1)
                    if gt >= NTILES and j == 0:
                        vector.wait_ge(sem_out, 16 * (gt - NTILES + 1))
                    vector.tensor_copy(
                        osbs[ti][:, j * 512 : (j + 1) * 512], psums[j][:]
                    ).then_inc(sem_cpb, 1)

    return nc


def _block_diag_weights(kernel: np.ndarray) -> np.ndarray:
    """kernel (1, CO, C, KT) -> (KT, 128, 128) block-diag lhsT.

    lhsT[k][ci + 32*g, co + 32*g] = kernel[0, co, ci, k]
    """
    wbd = np.zeros((KT, 128, 128), dtype=np.float32)
    wt = np.ascontiguousarray(kernel[0].transpose(2, 1, 0))  # (KT, C, CO)
    for g in range(NG):
        wbd[:, g * 32 : (g + 1) * 32, g * 32 : (g + 1) * 32] = wt
    return wbd


def _pad_x(x: np.ndarray) -> np.ndarray:
    xp = np.zeros((B, C, LP), dtype=np.float32)
    xp[:, :, :L] = x
    return xp


def kernel(x: np.ndarray, kernel: np.ndarray) -> np.ndarray:
    if "nc" not in _CACHE:
        _CACHE["nc"] = _build_nc()
    nc = _CACHE["nc"]

    wbd = _block_diag_weights(np.asarray(kernel, dtype=np.float32))
    xp = _pad_x(np.asarray(x, dtype=np.float32))

    in_maps = [
        {"x": xp[i * BPC : (i + 1) * BPC], "w": wbd} for i in range(NCORES)
    ]
    res = run_bass_kernel_spmd(nc, in_maps, list(range(NCORES)))
    full = np.concatenate([r["out"] for r in res.results], axis=0)
    return np.ascontiguousarray(full[:, :, :LOUT])
